# revision 2
# baseline (speedup 1.0000x reference)
"""Trainium2 Bass kernel v2 for nn_DiscriminatorBlock.

Design: 2x2 spatial classes (gh=y%2, gw=x%2). Partition p = 32*(2gh+gw)+c.
Host pre-permutes x into this layout (fp16, zero halos, contiguous) so DMA
uses large descriptors; host un-permutes the output. On-chip:
  conv0 (3x3 s1): dense central 128x128 matmul + dv 64x64 pair + du 8x32x32
    + 4 corner 32x32s per [4v x 128u] psum tile -> 4 PE slots/tile.
  c1 (6x6 s2 fused blur+conv+sqrt2) and rs (4x4 s2 fused blur+1x1+sqrt.5):
    16 (th,tw) shift matmuls vs class grid, K=128 always, M per shift
    multiplicity, thin shifts packed col-disjoint into shared slots.
Weight-stationary sweeps over psum tiles amortize LDWEIGHTS via
ldweights=False on repeat matmuls (REUSE flag).
"""
import sys
import os
sys.path.insert(0, '/opt/trn_rl_repo')
import numpy as np

H, W, C, F = 512, 512, 32, 32
N_CORES = 8
VS = 32                 # class-grid v rows per slab
T = (H // 2) // VS      # 8 slabs
UP = 264                # u pitch (256 data + 2+2 halo + pad)
XROWS = 262             # stored class-grid rows: v in [-2, 260)
REUSE = os.environ.get('V2_REUSE', '1') == '1'
SECTIONS = os.environ.get('V2_SECTIONS', 'c0,c1rs')
C0PASSES = [int(i) for i in os.environ.get('V2_C0PASSES', '01234')]
ORDER = os.environ.get('V2_ORDER', 'pass')
CHUNK0 = 4              # conv0 psum sweep width
CHUNK1 = 2              # c1/rs psum sweep width

SQ2 = float(np.sqrt(2.0))
SQH = float(np.sqrt(0.5))

OH_SETS = {-1: (0,), 0: (0, 1), 1: (0, 1), 2: (1,)}


def _runs(th, tw):
    cls = sorted(2 * oh + ow for oh in OH_SETS[th] for ow in OH_SETS[tw])
    rr = []
    s = p = cls[0]
    for c in cls[1:]:
        if c == p + 1:
            p = c
        else:
            rr.append((s, p + 1))
            s = p = c
    rr.append((s, p + 1))
    return rr


def _conv0_passes():
    p0 = [dict(k0=0, k1=4, m0=0, m1=4, dv=0, du=0)]
    p1 = [dict(k0=0, k1=2, m0=2, m1=4, dv=1, du=0),
          dict(k0=2, k1=4, m0=0, m1=2, dv=-1, du=0)]
    # du works: K cls (gh, gw=0)->M (mh, 1) for du=+1; (gh,1)->(mh,0) for -1.
    # Concurrent MMs need pairwise-distinct row groups AND col regions.
    def w(ki, ko, dv, du):
        return dict(k0=ki, k1=ki + 1, m0=ko, m1=ko + 1, dv=dv, du=du)
    p2a = [w(0, 1, 0, 1), w(2, 3, 0, 1), w(1, 0, 0, -1), w(3, 2, 0, -1)]
    p2b = [w(2, 1, 0, 1), w(0, 3, 0, 1), w(3, 0, 0, -1), w(1, 2, 0, -1)]
    p3 = []
    for du in (1, -1):
        for dv in (1, -1):
            gh = 0 if dv == 1 else 1
            gw = 0 if du == 1 else 1
            ki, ko = 2 * gh + gw, 2 * (1 - gh) + (1 - gw)
            p3.append(w(ki, ko, dv, du))
    # Order matters: p1 (dv pair, rows 0-63 + 64-127) row-conflicts with every
    # 32-row quad, serializing p2a from p2b; p3 reuses p2b's exact tile
    # positions (same-position serialization). This prevents concurrent
    # different-row-tile writes to the same PSUM region across passes.
    return [p0, p2a, p1, p2b, p3]


def _s2_passes():
    passes = [[dict(th=0, tw=0, m0=0, m1=4)],
              [dict(th=0, tw=1, m0=0, m1=4)],
              [dict(th=1, tw=0, m0=0, m1=4)],
              [dict(th=1, tw=1, m0=0, m1=4)]]
    for tw in (0, 1):
        passes.append([dict(th=-1, tw=tw, m0=0, m1=2),
                       dict(th=2, tw=tw, m0=2, m1=4)])
    for th in (0, 1):
        mm = []
        for tw in (-1, 2):
            for a, b in _runs(th, tw):
                mm.append(dict(th=th, tw=tw, m0=a, m1=b))
        passes.append(mm)
    corners = []
    for th in (-1, 2):
        for tw in (-1, 2):
            (a, b), = _runs(th, tw)
            corners.append(dict(th=th, tw=tw, m0=a, m1=b))
    passes.append(corners)
    return passes


_C0P = _conv0_passes()
_S2P = _s2_passes()


def _assign_cols():
    off = 0
    for passes in (_C0P, _S2P):
        for pas in passes:
            for mm in pas:
                mm['off'] = off
                off += 32 * (mm['m1'] - mm['m0'])
    # rs reuses _S2P geometry with its own columns
    rs = []
    for pas in _S2P:
        rp = []
        for mm in pas:
            m = dict(mm)
            m['off'] = off
            off += 32 * (mm['m1'] - mm['m0'])
            rp.append(m)
        rs.append(rp)
    return rs, off


_RSP, _WCOLS = _assign_cols()


def _pack_weights(w0, w1, w_res):
    bk = np.array([1.0, 3.0, 3.0, 1.0]) / 8.0
    W0e = w0.astype(np.float64) / np.sqrt(9 * 32)
    W1e = w1.astype(np.float64) / np.sqrt(9 * 32)
    WRe = w_res.astype(np.float64)[0, 0] / np.sqrt(32)
    K6 = np.zeros((6, 6, 32, 32))
    for r in range(3):
        for s in range(3):
            for a in range(4):
                for c in range(4):
                    K6[r + a, s + c] += bk[a] * bk[c] * W1e[r, s]
    K6 *= SQ2
    K4 = np.einsum('a,c,ij->acij', bk, bk, WRe) * SQH

    wts = np.zeros((128, _WCOLS), np.float16)
    for pas in _C0P:
        for mm in pas:
            for ki in range(mm['k0'], mm['k1']):
                gh, gw = ki >> 1, ki & 1
                for ko in range(mm['m0'], mm['m1']):
                    mh, mw = ko >> 1, ko & 1
                    dy = 2 * mm['dv'] + gh - mh
                    dx = 2 * mm['du'] + gw - mw
                    if abs(dy) <= 1 and abs(dx) <= 1:
                        wts[32 * ki:32 * ki + 32,
                            mm['off'] + 32 * (ko - mm['m0']):
                            mm['off'] + 32 * (ko - mm['m0']) + 32] = \
                            W0e[dy + 1, dx + 1].astype(np.float16)
    for passes, Kf, base in ((_S2P, K6, 2), (_RSP, K4, 1)):
        for pas in passes:
            for mm in pas:
                for ki in range(4):
                    gh, gw = ki >> 1, ki & 1
                    for ko in range(mm['m0'], mm['m1']):
                        oh, ow = ko >> 1, ko & 1
                        if mm['th'] - oh not in (-1, 0, 1):
                            continue
                        if mm['tw'] - ow not in (-1, 0, 1):
                            continue
                        r = 2 * (mm['th'] - oh) + gh + base
                        s = 2 * (mm['tw'] - ow) + gw + base
                        if 0 <= r < Kf.shape[0] and 0 <= s < Kf.shape[1]:
                            wts[32 * ki:32 * ki + 32,
                                mm['off'] + 32 * (ko - mm['m0']):
                                mm['off'] + 32 * (ko - mm['m0']) + 32] = \
                                Kf[r, s].astype(np.float16)
    return wts


def _build():
    import concourse.mybir as mybir
    from concourse import bacc
    from concourse.tile import TileContext

    F32 = mybir.dt.float32
    F16 = mybir.dt.float16
    ACTF = mybir.ActivationFunctionType

    nc = bacc.Bacc('TRN2', target_bir_lowering=False)
    x_t = nc.dram_tensor("x", [128, XROWS, UP], F16, kind="ExternalInput")
    w_t = nc.dram_tensor("wts", [128, _WCOLS], F16, kind="ExternalInput")
    b_t = nc.dram_tensor("biases", [128, 2], F32, kind="ExternalInput")
    o_t = nc.dram_tensor("out", [128, 128, 128], F32, kind="ExternalOutput")

    with TileContext(nc) as tc:
        with tc.tile_pool(name="const", bufs=1) as cpool, \
             tc.tile_pool(name="slab", bufs=2) as spool, \
             tc.tile_pool(name="stage", bufs=3) as gpool, \
             tc.tile_pool(name="ps0", bufs=1, space="PSUM") as pp0, \
             tc.tile_pool(name="ps1", bufs=1, space="PSUM") as pp1, \
             tc.tile_pool(name="ps2", bufs=1, space="PSUM") as pp2:

            wtile = cpool.tile([128, _WCOLS], F16, tag="wts")
            nc.sync.dma_start(out=wtile[:, :], in_=w_t[:, :])
            btile = cpool.tile([128, 2], F32, tag="bias")
            nc.sync.dma_start(out=btile[:, :], in_=b_t[:, :])

            slabs = {}

            def emit_load(t):
                v0 = VS * t
                xs = spool.tile([128, 38, UP], F16, tag="xs")
                nc.sync.dma_start(out=xs[:, :, :], in_=x_t[:, v0:v0 + 38, :])
                hs = spool.tile([128, 34, UP], F16, tag="hs")
                nc.gpsimd.memset(hs[:, :, 0:2], 0.0)
                nc.gpsimd.memset(hs[:, :, 258:UP], 0.0)
                if t == 0:
                    nc.gpsimd.memset(hs[:, 0:1, :], 0.0)
                if t == T - 1:
                    nc.gpsimd.memset(hs[:, 33:34, :], 0.0)
                if 'c0' not in SECTIONS:
                    nc.gpsimd.memset(hs[:, :, :], 0.0)
                slabs[t] = (xs, hs)

            def mm_c0(ps, xs, mm, hv, uh, start, stop, first):
                a = hv + 1 + mm['dv']
                b = 2 + 128 * uh + mm['du']
                k0, k1, m0, m1 = mm['k0'], mm['k1'], mm['m0'], mm['m1']
                inst = nc.tensor.matmul(
                    ps[32 * m0:32 * m1, :, :],
                    wtile[32 * k0:32 * k1,
                          mm['off']:mm['off'] + 32 * (m1 - m0)],
                    xs[32 * k0:32 * k1, a:a + 4, b:b + 128],
                    start=start, stop=stop,
                    tile_position=(32 * k0, 32 * m0), skip_group_check=True)
                if REUSE and not first:
                    inst.ins.ldweights = False

            def emit_conv0(t):
                xs, hs = slabs[t]
                lo = 1 if t == 0 else 0
                hi = 33 if t == T - 1 else 34
                tiles = []
                for hv in range(0, 36, 4):
                    w0v = max(lo - hv, 0)
                    w1v = min(hi - hv, 4)
                    if w1v <= w0v:
                        continue
                    for uh in (0, 1):
                        tiles.append((hv, uh, w0v, w1v))
                for ci in range(0, len(tiles), CHUNK0):
                    chunk = tiles[ci:ci + CHUNK0]
                    pss = [pp0.tile([128, 4, 128], F32, tag=f"c0b{i}",
                                    name=f"c0ps{i}") for i in range(len(chunk))]
                    if ORDER == 'pass':
                        for pi, pas in enumerate(_C0P):
                            if pi not in C0PASSES:
                                continue
                            last = pi == max(C0PASSES)
                            for ti, (hv, uh, _, _) in enumerate(chunk):
                                for mm in pas:
                                    mm_c0(pss[ti], xs, mm, hv, uh,
                                          start=(pi == 0), stop=last,
                                          first=(ti == 0))
                    else:
                        for ti, (hv, uh, _, _) in enumerate(chunk):
                            for pi, pas in enumerate(_C0P):
                                if pi not in C0PASSES:
                                    continue
                                last = pi == max(C0PASSES)
                                for mm in pas:
                                    mm_c0(pss[ti], xs, mm, hv, uh,
                                          start=(pi == 0), stop=last,
                                          first=True)
                    for ti, (hv, uh, w0v, w1v) in enumerate(chunk):
                        nc.scalar.activation(
                            hs[:, hv + w0v:hv + w1v,
                               2 + 128 * uh:2 + 128 * uh + 128],
                            pss[ti][:, w0v:w1v, :],
                            ACTF.Prelu, bias=btile[:, 0:1], alpha=0.2)

            def mm_s2(ps, src, passes, pi, mm, c, rowoff, start, stop, first):
                a = 8 * c + mm['th'] + rowoff
                b = 2 + mm['tw']
                m0, m1 = mm['m0'], mm['m1']
                inst = nc.tensor.matmul(
                    ps[32 * m0:32 * m1, :, :],
                    wtile[:, mm['off']:mm['off'] + 32 * (m1 - m0)],
                    src[:, a:a + 7:2, b:b + 255:2],
                    start=start, stop=stop,
                    tile_position=(0, 32 * m0), skip_group_check=True)
                if REUSE and not first:
                    inst.ins.ldweights = False

            def emit_c1rs(t):
                xs, hs = slabs[t]
                V0 = 16 * t
                for ci in range(0, 4, CHUNK1):
                    cs = list(range(ci, min(ci + CHUNK1, 4)))
                    ps1s = [pp1.tile([128, 4, 128], F32, tag=f"c1b{i}",
                                     name=f"c1ps{i}") for i in range(len(cs))]
                    ps2s = [pp2.tile([128, 4, 128], F32, tag=f"rsb{i}",
                                     name=f"rsps{i}") for i in range(len(cs))]
                    if ORDER == 'pass':
                        for pi, pas in enumerate(_S2P):
                            last = pi == len(_S2P) - 1
                            for ti, c in enumerate(cs):
                                for mm in pas:
                                    mm_s2(ps1s[ti], hs, _S2P, pi, mm, c, 1,
                                          start=(pi == 0), stop=last,
                                          first=(ti == 0))
                        for pi, pas in enumerate(_RSP):
                            last = pi == len(_RSP) - 1
                            for ti, c in enumerate(cs):
                                for mm in pas:
                                    mm_s2(ps2s[ti], xs, _RSP, pi, mm, c, 2,
                                          start=(pi == 0), stop=last,
                                          first=(ti == 0))
                    else:
                        for ti, c in enumerate(cs):
                            for pi, pas in enumerate(_S2P):
                                last = pi == len(_S2P) - 1
                                for mm in pas:
                                    mm_s2(ps1s[ti], hs, _S2P, pi, mm, c, 1,
                                          start=(pi == 0), stop=last,
                                          first=True)
                            for pi, pas in enumerate(_RSP):
                                last = pi == len(_RSP) - 1
                                for mm in pas:
                                    mm_s2(ps2s[ti], xs, _RSP, pi, mm, c, 2,
                                          start=(pi == 0), stop=last,
                                          first=True)
                    for ti, c in enumerate(cs):
                        h1sb = gpool.tile([128, 4, 128], F32, tag="h1sb")
                        nc.scalar.activation(h1sb[:, :, :], ps1s[ti][:, :, :],
                                             ACTF.Prelu, bias=btile[:, 1:2],
                                             alpha=0.2)
                        osum = gpool.tile([128, 4, 128], F32, tag="osum")
                        nc.vector.tensor_add(osum[:, :, :], h1sb[:, :, :],
                                             ps2s[ti][:, :, :])
                        nc.sync.dma_start(
                            out=o_t[:, V0 + 4 * c:V0 + 4 * c + 4, :],
                            in_=osum[:, :, :])

            emit_load(0)
            for t in range(T):
                if t + 1 < T:
                    emit_load(t + 1)
                if 'c0' in SECTIONS:
                    emit_conv0(t)
                if 'c1rs' in SECTIONS:
                    emit_c1rs(t)
                del slabs[t]

    nc.compile()
    return nc


_CACHE = {}
LAST_RESULTS = None


def _get_nc():
    if 'nc' not in _CACHE:
        _CACHE['nc'] = _build()
    return _CACHE['nc']


def _pack_x(xc):
    # xc [512, 512, 32] f32 -> [128, XROWS, UP] f16 class layout with halos
    xr = xc.reshape(256, 2, 256, 2, 32)            # v gh u gw c
    xr = np.ascontiguousarray(xr.transpose(1, 3, 4, 0, 2))  # gh gw c v u
    xr = xr.reshape(128, 256, 256).astype(np.float16)
    xp = np.zeros((128, XROWS, UP), np.float16)
    xp[:, 2:258, 2:258] = xr
    return xp


def kernel(x, w0, b0, w1, b1, w_res):
    from concourse.bass_utils import run_bass_kernel_spmd
    x = np.asarray(x, np.float32)
    wts = _pack_weights(np.asarray(w0), np.asarray(w1), np.asarray(w_res))
    biases = np.zeros((128, 2), np.float32)
    biases[:, 0] = np.tile(np.asarray(b0, np.float32), 4)
    biases[:, 1] = np.tile(np.asarray(b1, np.float32), 4)
    nc = _get_nc()
    in_maps = [{"x": _pack_x(x[i]), "wts": wts, "biases": biases}
               for i in range(N_CORES)]
    res = run_bass_kernel_spmd(nc, in_maps, core_ids=list(range(N_CORES)))
    global LAST_RESULTS
    LAST_RESULTS = res
    outs = []
    for i in range(N_CORES):
        o = res.results[i]["out"]                   # [128, 128, 128]
        o = o.reshape(2, 2, 32, 128, 128)           # oh ow f V U
        o = o.transpose(3, 0, 4, 1, 2).reshape(256, 256, 32)
        outs.append(o)
    return np.stack(outs).astype(np.float32)


# revision 3
# speedup vs baseline: 1.0170x; 1.0170x over previous
"""Trainium2 Bass kernel v2 for nn_DiscriminatorBlock.

Design: 2x2 spatial classes (gh=y%2, gw=x%2). Partition p = 32*(2gh+gw)+c.
Host pre-permutes x into this layout (fp16, zero halos, contiguous) so DMA
uses large descriptors; host un-permutes the output. On-chip:
  conv0 (3x3 s1): dense central 128x128 matmul + dv 64x64 pair + du 8x32x32
    + 4 corner 32x32s per [4v x 128u] psum tile -> 4 PE slots/tile.
  c1 (6x6 s2 fused blur+conv+sqrt2) and rs (4x4 s2 fused blur+1x1+sqrt.5):
    16 (th,tw) shift matmuls vs class grid, K=128 always, M per shift
    multiplicity, thin shifts packed col-disjoint into shared slots.
Weight-stationary sweeps over psum tiles amortize LDWEIGHTS via
ldweights=False on repeat matmuls (REUSE flag).
"""
import sys
import os
sys.path.insert(0, '/opt/trn_rl_repo')
import numpy as np

H, W, C, F = 512, 512, 32, 32
N_CORES = 8
VS = 32                 # class-grid v rows per slab
T = (H // 2) // VS      # 8 slabs
UP = 264                # u pitch (256 data + 2+2 halo + pad)
XROWS = 262             # stored class-grid rows: v in [-2, 260)
REUSE = os.environ.get('V2_REUSE', '1') == '1'
SECTIONS = os.environ.get('V2_SECTIONS', 'c0,c1rs')
C0PASSES = [int(i) for i in os.environ.get('V2_C0PASSES', '0123')]
ORDER = os.environ.get('V2_ORDER', 'pass')
CHUNK0 = 4              # conv0 psum sweep width
CHUNK1 = 2              # c1/rs psum sweep width

SQ2 = float(np.sqrt(2.0))
SQH = float(np.sqrt(0.5))

OH_SETS = {-1: (0,), 0: (0, 1), 1: (0, 1), 2: (1,)}


def _runs(th, tw):
    cls = sorted(2 * oh + ow for oh in OH_SETS[th] for ow in OH_SETS[tw])
    rr = []
    s = p = cls[0]
    for c in cls[1:]:
        if c == p + 1:
            p = c
        else:
            rr.append((s, p + 1))
            s = p = c
    rr.append((s, p + 1))
    return rr


def _conv0_passes():
    # src 'x': plain class slab. src 'e': companion slab with u-shift baked in
    # per row group (even classes u+1, odd classes u-1), so all 8 du edge
    # works collapse into one dense full-K matmul (pE) and the 4 corners into
    # one 64x64 pair (pC). No 32x32 quads -> fewer slots, no cross-pass
    # same-psum-region races (all passes full-K or the proven 2D-disjoint
    # pair; pC reuses p1's tile positions so they serialize per-position).
    p0 = [dict(k0=0, k1=4, m0=0, m1=4, dv=0, du=0, src='x')]
    pE = [dict(k0=0, k1=4, m0=0, m1=4, dv=0, du=0, src='e')]
    p1 = [dict(k0=0, k1=2, m0=2, m1=4, dv=1, du=0, src='x'),
          dict(k0=2, k1=4, m0=0, m1=2, dv=-1, du=0, src='x')]
    pC = [dict(k0=0, k1=2, m0=2, m1=4, dv=1, du=0, src='e'),
          dict(k0=2, k1=4, m0=0, m1=2, dv=-1, du=0, src='e')]
    return [p0, pE, p1, pC]


def _s2_passes():
    passes = [[dict(th=0, tw=0, m0=0, m1=4)],
              [dict(th=0, tw=1, m0=0, m1=4)],
              [dict(th=1, tw=0, m0=0, m1=4)],
              [dict(th=1, tw=1, m0=0, m1=4)]]
    for tw in (0, 1):
        passes.append([dict(th=-1, tw=tw, m0=0, m1=2),
                       dict(th=2, tw=tw, m0=2, m1=4)])
    for th in (0, 1):
        mm = []
        for tw in (-1, 2):
            for a, b in _runs(th, tw):
                mm.append(dict(th=th, tw=tw, m0=a, m1=b))
        passes.append(mm)
    corners = []
    for th in (-1, 2):
        for tw in (-1, 2):
            (a, b), = _runs(th, tw)
            corners.append(dict(th=th, tw=tw, m0=a, m1=b))
    passes.append(corners)
    return passes


_C0P = _conv0_passes()
_S2P = _s2_passes()


def _assign_cols():
    off = 0
    for passes in (_C0P, _S2P):
        for pas in passes:
            for mm in pas:
                mm['off'] = off
                off += 32 * (mm['m1'] - mm['m0'])
    # rs reuses _S2P geometry with its own columns
    rs = []
    for pas in _S2P:
        rp = []
        for mm in pas:
            m = dict(mm)
            m['off'] = off
            off += 32 * (mm['m1'] - mm['m0'])
            rp.append(m)
        rs.append(rp)
    return rs, off


_RSP, _WCOLS = _assign_cols()


def _pack_weights(w0, w1, w_res):
    bk = np.array([1.0, 3.0, 3.0, 1.0]) / 8.0
    W0e = w0.astype(np.float64) / np.sqrt(9 * 32)
    W1e = w1.astype(np.float64) / np.sqrt(9 * 32)
    WRe = w_res.astype(np.float64)[0, 0] / np.sqrt(32)
    K6 = np.zeros((6, 6, 32, 32))
    for r in range(3):
        for s in range(3):
            for a in range(4):
                for c in range(4):
                    K6[r + a, s + c] += bk[a] * bk[c] * W1e[r, s]
    K6 *= SQ2
    K4 = np.einsum('a,c,ij->acij', bk, bk, WRe) * SQH

    wts = np.zeros((128, _WCOLS), np.float16)
    for pas in _C0P:
        for mm in pas:
            for ki in range(mm['k0'], mm['k1']):
                gh, gw = ki >> 1, ki & 1
                for ko in range(mm['m0'], mm['m1']):
                    mh, mw = ko >> 1, ko & 1
                    du_eff = mm['du'] if mm.get('src') != 'e' else \
                        (1 if gw == 0 else -1)
                    dy = 2 * mm['dv'] + gh - mh
                    dx = 2 * du_eff + gw - mw
                    if abs(dy) <= 1 and abs(dx) <= 1:
                        wts[32 * ki:32 * ki + 32,
                            mm['off'] + 32 * (ko - mm['m0']):
                            mm['off'] + 32 * (ko - mm['m0']) + 32] = \
                            W0e[dy + 1, dx + 1].astype(np.float16)
    for passes, Kf, base in ((_S2P, K6, 2), (_RSP, K4, 1)):
        for pas in passes:
            for mm in pas:
                for ki in range(4):
                    gh, gw = ki >> 1, ki & 1
                    for ko in range(mm['m0'], mm['m1']):
                        oh, ow = ko >> 1, ko & 1
                        if mm['th'] - oh not in (-1, 0, 1):
                            continue
                        if mm['tw'] - ow not in (-1, 0, 1):
                            continue
                        r = 2 * (mm['th'] - oh) + gh + base
                        s = 2 * (mm['tw'] - ow) + gw + base
                        if 0 <= r < Kf.shape[0] and 0 <= s < Kf.shape[1]:
                            wts[32 * ki:32 * ki + 32,
                                mm['off'] + 32 * (ko - mm['m0']):
                                mm['off'] + 32 * (ko - mm['m0']) + 32] = \
                                Kf[r, s].astype(np.float16)
    return wts


def _build():
    import concourse.mybir as mybir
    from concourse import bacc
    from concourse.tile import TileContext

    F32 = mybir.dt.float32
    F16 = mybir.dt.float16
    ACTF = mybir.ActivationFunctionType

    nc = bacc.Bacc('TRN2', target_bir_lowering=False)
    x_t = nc.dram_tensor("x", [128, XROWS, UP], F16, kind="ExternalInput")
    xe_t = nc.dram_tensor("xe", [128, XROWS, UP], F16, kind="ExternalInput")
    w_t = nc.dram_tensor("wts", [128, _WCOLS], F16, kind="ExternalInput")
    b_t = nc.dram_tensor("biases", [128, 2], F32, kind="ExternalInput")
    o_t = nc.dram_tensor("out", [128, 128, 128], F32, kind="ExternalOutput")

    with TileContext(nc) as tc:
        with tc.tile_pool(name="const", bufs=1) as cpool, \
             tc.tile_pool(name="slab", bufs=2) as spool, \
             tc.tile_pool(name="stage", bufs=3) as gpool, \
             tc.tile_pool(name="ps0", bufs=1, space="PSUM") as pp0, \
             tc.tile_pool(name="ps1", bufs=1, space="PSUM") as pp1, \
             tc.tile_pool(name="ps2", bufs=1, space="PSUM") as pp2:

            wtile = cpool.tile([128, _WCOLS], F16, tag="wts")
            nc.sync.dma_start(out=wtile[:, :], in_=w_t[:, :])
            btile = cpool.tile([128, 2], F32, tag="bias")
            nc.sync.dma_start(out=btile[:, :], in_=b_t[:, :])

            slabs = {}

            def emit_load(t):
                v0 = VS * t
                xs = spool.tile([128, 38, UP], F16, tag="xs")
                nc.sync.dma_start(out=xs[:, :, :], in_=x_t[:, v0:v0 + 38, :])
                xe = spool.tile([128, 38, UP], F16, tag="xe")
                nc.sync.dma_start(out=xe[:, :, :], in_=xe_t[:, v0:v0 + 38, :])
                hs = spool.tile([128, 34, UP], F16, tag="hs")
                nc.gpsimd.memset(hs[:, :, 0:2], 0.0)
                nc.gpsimd.memset(hs[:, :, 258:UP], 0.0)
                if t == 0:
                    nc.gpsimd.memset(hs[:, 0:1, :], 0.0)
                if t == T - 1:
                    nc.gpsimd.memset(hs[:, 33:34, :], 0.0)
                if 'c0' not in SECTIONS:
                    nc.gpsimd.memset(hs[:, :, :], 0.0)
                slabs[t] = (xs, xe, hs)

            def mm_c0(ps, srcs, mm, hv, uh, start, stop, first):
                xsrc = srcs[0] if mm.get('src') != 'e' else srcs[1]
                a = hv + 1 + mm['dv']
                b = 2 + 128 * uh + mm['du']
                k0, k1, m0, m1 = mm['k0'], mm['k1'], mm['m0'], mm['m1']
                inst = nc.tensor.matmul(
                    ps[32 * m0:32 * m1, :, :],
                    wtile[32 * k0:32 * k1,
                          mm['off']:mm['off'] + 32 * (m1 - m0)],
                    xsrc[32 * k0:32 * k1, a:a + 4, b:b + 128],
                    start=start, stop=stop,
                    tile_position=(32 * k0, 32 * m0), skip_group_check=True)
                if REUSE and not first:
                    inst.ins.ldweights = False

            def emit_conv0(t):
                xs, xe, hs = slabs[t]
                lo = 1 if t == 0 else 0
                hi = 33 if t == T - 1 else 34
                tiles = []
                for hv in range(0, 36, 4):
                    w0v = max(lo - hv, 0)
                    w1v = min(hi - hv, 4)
                    if w1v <= w0v:
                        continue
                    for uh in (0, 1):
                        tiles.append((hv, uh, w0v, w1v))
                for ci in range(0, len(tiles), CHUNK0):
                    chunk = tiles[ci:ci + CHUNK0]
                    pss = [pp0.tile([128, 4, 128], F32, tag=f"c0b{i}",
                                    name=f"c0ps{i}") for i in range(len(chunk))]
                    if ORDER == 'pass':
                        for pi, pas in enumerate(_C0P):
                            if pi not in C0PASSES:
                                continue
                            last = pi == max(C0PASSES)
                            for ti, (hv, uh, _, _) in enumerate(chunk):
                                for mm in pas:
                                    mm_c0(pss[ti], (xs, xe), mm, hv, uh,
                                          start=(pi == 0), stop=last,
                                          first=(ti == 0))
                    else:
                        for ti, (hv, uh, _, _) in enumerate(chunk):
                            for pi, pas in enumerate(_C0P):
                                if pi not in C0PASSES:
                                    continue
                                last = pi == max(C0PASSES)
                                for mm in pas:
                                    mm_c0(pss[ti], (xs, xe), mm, hv, uh,
                                          start=(pi == 0), stop=last,
                                          first=True)
                    for ti, (hv, uh, w0v, w1v) in enumerate(chunk):
                        nc.scalar.activation(
                            hs[:, hv + w0v:hv + w1v,
                               2 + 128 * uh:2 + 128 * uh + 128],
                            pss[ti][:, w0v:w1v, :],
                            ACTF.Prelu, bias=btile[:, 0:1], alpha=0.2)

            def mm_s2(ps, src, passes, pi, mm, c, rowoff, start, stop, first):
                a = 8 * c + mm['th'] + rowoff
                b = 2 + mm['tw']
                m0, m1 = mm['m0'], mm['m1']
                inst = nc.tensor.matmul(
                    ps[32 * m0:32 * m1, :, :],
                    wtile[:, mm['off']:mm['off'] + 32 * (m1 - m0)],
                    src[:, a:a + 7:2, b:b + 255:2],
                    start=start, stop=stop,
                    tile_position=(0, 32 * m0), skip_group_check=True)
                if REUSE and not first:
                    inst.ins.ldweights = False

            def emit_c1rs(t):
                xs, xe, hs = slabs[t]
                V0 = 16 * t
                for ci in range(0, 4, CHUNK1):
                    cs = list(range(ci, min(ci + CHUNK1, 4)))
                    ps1s = [pp1.tile([128, 4, 128], F32, tag=f"c1b{i}",
                                     name=f"c1ps{i}") for i in range(len(cs))]
                    ps2s = [pp2.tile([128, 4, 128], F32, tag=f"rsb{i}",
                                     name=f"rsps{i}") for i in range(len(cs))]
                    if ORDER == 'pass':
                        for pi, pas in enumerate(_S2P):
                            last = pi == len(_S2P) - 1
                            for ti, c in enumerate(cs):
                                for mm in pas:
                                    mm_s2(ps1s[ti], hs, _S2P, pi, mm, c, 1,
                                          start=(pi == 0), stop=last,
                                          first=(ti == 0))
                        for pi, pas in enumerate(_RSP):
                            last = pi == len(_RSP) - 1
                            for ti, c in enumerate(cs):
                                for mm in pas:
                                    mm_s2(ps2s[ti], xs, _RSP, pi, mm, c, 2,
                                          start=(pi == 0), stop=last,
                                          first=(ti == 0))
                    else:
                        for ti, c in enumerate(cs):
                            for pi, pas in enumerate(_S2P):
                                last = pi == len(_S2P) - 1
                                for mm in pas:
                                    mm_s2(ps1s[ti], hs, _S2P, pi, mm, c, 1,
                                          start=(pi == 0), stop=last,
                                          first=True)
                            for pi, pas in enumerate(_RSP):
                                last = pi == len(_RSP) - 1
                                for mm in pas:
                                    mm_s2(ps2s[ti], xs, _RSP, pi, mm, c, 2,
                                          start=(pi == 0), stop=last,
                                          first=True)
                    for ti, c in enumerate(cs):
                        h1sb = gpool.tile([128, 4, 128], F32, tag="h1sb")
                        nc.scalar.activation(h1sb[:, :, :], ps1s[ti][:, :, :],
                                             ACTF.Prelu, bias=btile[:, 1:2],
                                             alpha=0.2)
                        osum = gpool.tile([128, 4, 128], F32, tag="osum")
                        nc.vector.tensor_add(osum[:, :, :], h1sb[:, :, :],
                                             ps2s[ti][:, :, :])
                        nc.sync.dma_start(
                            out=o_t[:, V0 + 4 * c:V0 + 4 * c + 4, :],
                            in_=osum[:, :, :])

            emit_load(0)
            for t in range(T):
                if t + 1 < T:
                    emit_load(t + 1)
                if 'c0' in SECTIONS:
                    emit_conv0(t)
                if 'c1rs' in SECTIONS:
                    emit_c1rs(t)
                del slabs[t]

    nc.compile()
    return nc


_CACHE = {}
LAST_RESULTS = None


def _get_nc():
    if 'nc' not in _CACHE:
        _CACHE['nc'] = _build()
    return _CACHE['nc']


def _pack_x(xc):
    # xc [512, 512, 32] f32 -> two [128, XROWS, UP] f16 class-layout slabs:
    # xp (plain, with halos) and xe (per-row-group u-shift companion:
    # even classes (gw=0) shifted u+1, odd classes (gw=1) shifted u-1).
    xr = xc.reshape(256, 2, 256, 2, 32)            # v gh u gw c
    xr = np.ascontiguousarray(xr.transpose(1, 3, 4, 0, 2))  # gh gw c v u
    xr = xr.reshape(128, 256, 256).astype(np.float16)
    xp = np.zeros((128, XROWS, UP), np.float16)
    xp[:, 2:258, 2:258] = xr
    xe = np.zeros_like(xp)
    even = np.zeros(128, bool)
    for a in range(4):
        if a % 2 == 0:
            even[32 * a:32 * a + 32] = True
    xe[even, :, 0:UP - 1] = xp[even, :, 1:UP]
    xe[~even, :, 1:UP] = xp[~even, :, 0:UP - 1]
    return xp, xe


def kernel(x, w0, b0, w1, b1, w_res):
    from concourse.bass_utils import run_bass_kernel_spmd
    x = np.asarray(x, np.float32)
    wts = _pack_weights(np.asarray(w0), np.asarray(w1), np.asarray(w_res))
    biases = np.zeros((128, 2), np.float32)
    biases[:, 0] = np.tile(np.asarray(b0, np.float32), 4)
    biases[:, 1] = np.tile(np.asarray(b1, np.float32), 4)
    nc = _get_nc()
    in_maps = []
    for i in range(N_CORES):
        xp, xe = _pack_x(x[i])
        in_maps.append({"x": xp, "xe": xe, "wts": wts, "biases": biases})
    res = run_bass_kernel_spmd(nc, in_maps, core_ids=list(range(N_CORES)))
    global LAST_RESULTS
    LAST_RESULTS = res
    outs = []
    for i in range(N_CORES):
        o = res.results[i]["out"]                   # [128, 128, 128]
        o = o.reshape(2, 2, 32, 128, 128)           # oh ow f V U
        o = o.transpose(3, 0, 4, 1, 2).reshape(256, 256, 32)
        outs.append(o)
    return np.stack(outs).astype(np.float32)


# revision 4
# speedup vs baseline: 1.0511x; 1.0335x over previous
"""Trainium2 Bass kernel v2 for nn_DiscriminatorBlock.

Design: 2x2 spatial classes (gh=y%2, gw=x%2). Partition p = 32*(2gh+gw)+c.
Host pre-permutes x into this layout (fp16, zero halos, contiguous) so DMA
uses large descriptors; host un-permutes the output. On-chip:
  conv0 (3x3 s1): dense central 128x128 matmul + dv 64x64 pair + du 8x32x32
    + 4 corner 32x32s per [4v x 128u] psum tile -> 4 PE slots/tile.
  c1 (6x6 s2 fused blur+conv+sqrt2) and rs (4x4 s2 fused blur+1x1+sqrt.5):
    16 (th,tw) shift matmuls vs class grid, K=128 always, M per shift
    multiplicity, thin shifts packed col-disjoint into shared slots.
Weight-stationary sweeps over psum tiles amortize LDWEIGHTS via
ldweights=False on repeat matmuls (REUSE flag).
"""
import sys
import os
sys.path.insert(0, '/opt/trn_rl_repo')
import numpy as np

H, W, C, F = 512, 512, 32, 32
N_CORES = 8
VS = 32                 # class-grid v rows per slab
T = (H // 2) // VS      # 8 slabs
UP = 264                # u pitch (256 data + 2+2 halo + pad)
XROWS = 262             # stored class-grid rows: v in [-2, 260)
REUSE = os.environ.get('V2_REUSE', '1') == '1'
SECTIONS = os.environ.get('V2_SECTIONS', 'c0,c1rs')
C0PASSES = [int(i) for i in os.environ.get('V2_C0PASSES', '0123')]
ORDER = os.environ.get('V2_ORDER', 'pass')
CHUNK0 = 4              # conv0 psum sweep width
CHUNK1 = 2              # c1/rs psum sweep width

SQ2 = float(np.sqrt(2.0))
SQH = float(np.sqrt(0.5))

OH_SETS = {-1: (0,), 0: (0, 1), 1: (0, 1), 2: (1,)}


def _runs(th, tw):
    cls = sorted(2 * oh + ow for oh in OH_SETS[th] for ow in OH_SETS[tw])
    rr = []
    s = p = cls[0]
    for c in cls[1:]:
        if c == p + 1:
            p = c
        else:
            rr.append((s, p + 1))
            s = p = c
    rr.append((s, p + 1))
    return rr


def _conv0_passes():
    # src 'x': plain class slab. src 'e': companion slab with u-shift baked in
    # per row group (even classes u+1, odd classes u-1), so all 8 du edge
    # works collapse into one dense full-K matmul (pE) and the 4 corners into
    # one 64x64 pair (pC). No 32x32 quads -> fewer slots, no cross-pass
    # same-psum-region races (all passes full-K or the proven 2D-disjoint
    # pair; pC reuses p1's tile positions so they serialize per-position).
    p0 = [dict(k0=0, k1=4, m0=0, m1=4, dv=0, du=0, src='x')]
    pE = [dict(k0=0, k1=4, m0=0, m1=4, dv=0, du=0, src='e')]
    p1 = [dict(k0=0, k1=2, m0=2, m1=4, dv=1, du=0, src='x'),
          dict(k0=2, k1=4, m0=0, m1=2, dv=-1, du=0, src='x')]
    pC = [dict(k0=0, k1=2, m0=2, m1=4, dv=1, du=0, src='e'),
          dict(k0=2, k1=4, m0=0, m1=2, dv=-1, du=0, src='e')]
    return [p0, pE, p1, pC]


def _s2_passes():
    passes = [[dict(th=0, tw=0, m0=0, m1=4)],
              [dict(th=0, tw=1, m0=0, m1=4)],
              [dict(th=1, tw=0, m0=0, m1=4)],
              [dict(th=1, tw=1, m0=0, m1=4)]]
    for tw in (0, 1):
        passes.append([dict(th=-1, tw=tw, m0=0, m1=2),
                       dict(th=2, tw=tw, m0=2, m1=4)])
    for th in (0, 1):
        mm = []
        for tw in (-1, 2):
            for a, b in _runs(th, tw):
                mm.append(dict(th=th, tw=tw, m0=a, m1=b))
        passes.append(mm)
    corners = []
    for th in (-1, 2):
        for tw in (-1, 2):
            (a, b), = _runs(th, tw)
            corners.append(dict(th=th, tw=tw, m0=a, m1=b))
    passes.append(corners)
    return passes


_C0P = _conv0_passes()
_S2P = _s2_passes()


def _assign_cols():
    off = 0
    for passes in (_C0P, _S2P):
        for pas in passes:
            for mm in pas:
                mm['off'] = off
                off += 32 * (mm['m1'] - mm['m0'])
    # rs reuses _S2P geometry with its own columns
    rs = []
    for pas in _S2P:
        rp = []
        for mm in pas:
            m = dict(mm)
            m['off'] = off
            off += 32 * (mm['m1'] - mm['m0'])
            rp.append(m)
        rs.append(rp)
    return rs, off


_RSP, _WCOLS = _assign_cols()


def _pack_weights(w0, w1, w_res):
    bk = np.array([1.0, 3.0, 3.0, 1.0]) / 8.0
    W0e = w0.astype(np.float64) / np.sqrt(9 * 32)
    W1e = w1.astype(np.float64) / np.sqrt(9 * 32)
    WRe = w_res.astype(np.float64)[0, 0] / np.sqrt(32)
    K6 = np.zeros((6, 6, 32, 32))
    for r in range(3):
        for s in range(3):
            for a in range(4):
                for c in range(4):
                    K6[r + a, s + c] += bk[a] * bk[c] * W1e[r, s]
    K6 *= SQ2
    K4 = np.einsum('a,c,ij->acij', bk, bk, WRe) * SQH

    wts = np.zeros((128, _WCOLS), np.float16)
    for pas in _C0P:
        for mm in pas:
            for ki in range(mm['k0'], mm['k1']):
                gh, gw = ki >> 1, ki & 1
                for ko in range(mm['m0'], mm['m1']):
                    mh, mw = ko >> 1, ko & 1
                    du_eff = mm['du'] if mm.get('src') != 'e' else \
                        (1 if gw == 0 else -1)
                    dy = 2 * mm['dv'] + gh - mh
                    dx = 2 * du_eff + gw - mw
                    if abs(dy) <= 1 and abs(dx) <= 1:
                        wts[32 * ki:32 * ki + 32,
                            mm['off'] + 32 * (ko - mm['m0']):
                            mm['off'] + 32 * (ko - mm['m0']) + 32] = \
                            W0e[dy + 1, dx + 1].astype(np.float16)
    for passes, Kf, base in ((_S2P, K6, 2), (_RSP, K4, 1)):
        for pas in passes:
            for mm in pas:
                for ki in range(4):
                    gh, gw = ki >> 1, ki & 1
                    for ko in range(mm['m0'], mm['m1']):
                        oh, ow = ko >> 1, ko & 1
                        if mm['th'] - oh not in (-1, 0, 1):
                            continue
                        if mm['tw'] - ow not in (-1, 0, 1):
                            continue
                        r = 2 * (mm['th'] - oh) + gh + base
                        s = 2 * (mm['tw'] - ow) + gw + base
                        if 0 <= r < Kf.shape[0] and 0 <= s < Kf.shape[1]:
                            wts[32 * ki:32 * ki + 32,
                                mm['off'] + 32 * (ko - mm['m0']):
                                mm['off'] + 32 * (ko - mm['m0']) + 32] = \
                                Kf[r, s].astype(np.float16)
    return wts


def _build():
    import concourse.mybir as mybir
    from concourse import bacc
    from concourse.tile import TileContext

    F32 = mybir.dt.float32
    F16 = mybir.dt.float16
    ACTF = mybir.ActivationFunctionType

    nc = bacc.Bacc('TRN2', target_bir_lowering=False)
    x_t = nc.dram_tensor("x", [128, XROWS, UP], F16, kind="ExternalInput")
    xe_t = nc.dram_tensor("xe", [128, XROWS, UP], F16, kind="ExternalInput")
    w_t = nc.dram_tensor("wts", [128, _WCOLS], F16, kind="ExternalInput")
    b_t = nc.dram_tensor("biases", [128, 2], F32, kind="ExternalInput")
    o_t = nc.dram_tensor("out", [128, 128, 128], F32, kind="ExternalOutput")

    with TileContext(nc) as tc:
        with tc.tile_pool(name="const", bufs=1) as cpool, \
             tc.tile_pool(name="slab", bufs=2) as spool, \
             tc.tile_pool(name="stage", bufs=3) as gpool, \
             tc.tile_pool(name="ps0", bufs=1, space="PSUM") as pp0, \
             tc.tile_pool(name="ps1", bufs=1, space="PSUM") as pp1, \
             tc.tile_pool(name="ps2", bufs=1, space="PSUM") as pp2:

            wtile = cpool.tile([128, _WCOLS], F16, tag="wts")
            # conv0 weights (low cols) land first so the first chunk starts
            nc.sync.dma_start(out=wtile[:, 0:640], in_=w_t[:, 0:640])
            nc.sync.dma_start(out=wtile[:, 640:_WCOLS], in_=w_t[:, 640:_WCOLS])
            btile = cpool.tile([128, 2], F32, tag="bias")
            nc.sync.dma_start(out=btile[:, :], in_=b_t[:, :])

            slabs = {}

            def emit_load(t):
                v0 = VS * t
                xs = spool.tile([128, 38, UP], F16, tag="xs")
                xe = spool.tile([128, 38, UP], F16, tag="xe")
                if t == 0:
                    # split the cold-start load so the first conv0 chunk can
                    # begin after ~10 rows instead of the full 38
                    for r0, r1 in ((0, 10), (10, 20), (20, 29), (29, 38)):
                        nc.sync.dma_start(out=xs[:, r0:r1, :],
                                          in_=x_t[:, v0 + r0:v0 + r1, :])
                        nc.sync.dma_start(out=xe[:, r0:r1, :],
                                          in_=xe_t[:, v0 + r0:v0 + r1, :])
                else:
                    nc.sync.dma_start(out=xs[:, :, :], in_=x_t[:, v0:v0 + 38, :])
                    nc.sync.dma_start(out=xe[:, :, :], in_=xe_t[:, v0:v0 + 38, :])
                hs = spool.tile([128, 34, UP], F16, tag="hs")
                nc.gpsimd.memset(hs[:, :, 0:2], 0.0)
                nc.gpsimd.memset(hs[:, :, 258:UP], 0.0)
                if t == 0:
                    nc.gpsimd.memset(hs[:, 0:1, :], 0.0)
                if t == T - 1:
                    nc.gpsimd.memset(hs[:, 33:34, :], 0.0)
                if 'c0' not in SECTIONS:
                    nc.gpsimd.memset(hs[:, :, :], 0.0)
                slabs[t] = (xs, xe, hs)

            def mm_c0(ps, srcs, mm, hv, uh, start, stop, first):
                xsrc = srcs[0] if mm.get('src') != 'e' else srcs[1]
                a = hv + 1 + mm['dv']
                b = 2 + 128 * uh + mm['du']
                k0, k1, m0, m1 = mm['k0'], mm['k1'], mm['m0'], mm['m1']
                inst = nc.tensor.matmul(
                    ps[32 * m0:32 * m1, :, :],
                    wtile[32 * k0:32 * k1,
                          mm['off']:mm['off'] + 32 * (m1 - m0)],
                    xsrc[32 * k0:32 * k1, a:a + 4, b:b + 128],
                    start=start, stop=stop,
                    tile_position=(32 * k0, 32 * m0), skip_group_check=True)
                if REUSE and not first:
                    inst.ins.ldweights = False

            def emit_conv0(t):
                xs, xe, hs = slabs[t]
                lo = 1 if t == 0 else 0
                hi = 33 if t == T - 1 else 34
                tiles = []
                for hv in range(0, 36, 4):
                    w0v = max(lo - hv, 0)
                    w1v = min(hi - hv, 4)
                    if w1v <= w0v:
                        continue
                    for uh in (0, 1):
                        tiles.append((hv, uh, w0v, w1v))
                for ci in range(0, len(tiles), CHUNK0):
                    chunk = tiles[ci:ci + CHUNK0]
                    pss = [pp0.tile([128, 4, 128], F32, tag=f"c0b{i}",
                                    name=f"c0ps{i}") for i in range(len(chunk))]
                    if ORDER == 'pass':
                        for pi, pas in enumerate(_C0P):
                            if pi not in C0PASSES:
                                continue
                            last = pi == max(C0PASSES)
                            for ti, (hv, uh, _, _) in enumerate(chunk):
                                for mm in pas:
                                    mm_c0(pss[ti], (xs, xe), mm, hv, uh,
                                          start=(pi == 0), stop=last,
                                          first=(ti == 0))
                    else:
                        for ti, (hv, uh, _, _) in enumerate(chunk):
                            for pi, pas in enumerate(_C0P):
                                if pi not in C0PASSES:
                                    continue
                                last = pi == max(C0PASSES)
                                for mm in pas:
                                    mm_c0(pss[ti], (xs, xe), mm, hv, uh,
                                          start=(pi == 0), stop=last,
                                          first=True)
                    for ti, (hv, uh, w0v, w1v) in enumerate(chunk):
                        nc.scalar.activation(
                            hs[:, hv + w0v:hv + w1v,
                               2 + 128 * uh:2 + 128 * uh + 128],
                            pss[ti][:, w0v:w1v, :],
                            ACTF.Prelu, bias=btile[:, 0:1], alpha=0.2)

            def mm_s2(ps, src, passes, pi, mm, c, rowoff, start, stop, first):
                a = 8 * c + mm['th'] + rowoff
                b = 2 + mm['tw']
                m0, m1 = mm['m0'], mm['m1']
                inst = nc.tensor.matmul(
                    ps[32 * m0:32 * m1, :, :],
                    wtile[:, mm['off']:mm['off'] + 32 * (m1 - m0)],
                    src[:, a:a + 7:2, b:b + 255:2],
                    start=start, stop=stop,
                    tile_position=(0, 32 * m0), skip_group_check=True)
                if REUSE and not first:
                    inst.ins.ldweights = False

            def emit_c1rs(t):
                xs, xe, hs = slabs[t]
                V0 = 16 * t
                for ci in range(0, 4, CHUNK1):
                    cs = list(range(ci, min(ci + CHUNK1, 4)))
                    ps1s = [pp1.tile([128, 4, 128], F32, tag=f"c1b{i}",
                                     name=f"c1ps{i}") for i in range(len(cs))]
                    ps2s = [pp2.tile([128, 4, 128], F32, tag=f"rsb{i}",
                                     name=f"rsps{i}") for i in range(len(cs))]
                    if ORDER == 'pass':
                        for pi, pas in enumerate(_S2P):
                            last = pi == len(_S2P) - 1
                            for ti, c in enumerate(cs):
                                for mm in pas:
                                    mm_s2(ps1s[ti], hs, _S2P, pi, mm, c, 1,
                                          start=(pi == 0), stop=last,
                                          first=(ti == 0))
                        for pi, pas in enumerate(_RSP):
                            last = pi == len(_RSP) - 1
                            for ti, c in enumerate(cs):
                                for mm in pas:
                                    mm_s2(ps2s[ti], xs, _RSP, pi, mm, c, 2,
                                          start=(pi == 0), stop=last,
                                          first=(ti == 0))
                    else:
                        for ti, c in enumerate(cs):
                            for pi, pas in enumerate(_S2P):
                                last = pi == len(_S2P) - 1
                                for mm in pas:
                                    mm_s2(ps1s[ti], hs, _S2P, pi, mm, c, 1,
                                          start=(pi == 0), stop=last,
                                          first=True)
                            for pi, pas in enumerate(_RSP):
                                last = pi == len(_RSP) - 1
                                for mm in pas:
                                    mm_s2(ps2s[ti], xs, _RSP, pi, mm, c, 2,
                                          start=(pi == 0), stop=last,
                                          first=True)
                    for ti, c in enumerate(cs):
                        h1sb = gpool.tile([128, 4, 128], F32, tag="h1sb")
                        nc.scalar.activation(h1sb[:, :, :], ps1s[ti][:, :, :],
                                             ACTF.Prelu, bias=btile[:, 1:2],
                                             alpha=0.2)
                        osum = gpool.tile([128, 4, 128], F32, tag="osum")
                        nc.vector.tensor_add(osum[:, :, :], h1sb[:, :, :],
                                             ps2s[ti][:, :, :])
                        nc.sync.dma_start(
                            out=o_t[:, V0 + 4 * c:V0 + 4 * c + 4, :],
                            in_=osum[:, :, :])

            emit_load(0)
            for t in range(T):
                if t + 1 < T:
                    emit_load(t + 1)
                if 'c0' in SECTIONS:
                    emit_conv0(t)
                if 'c1rs' in SECTIONS:
                    emit_c1rs(t)
                del slabs[t]

    nc.compile()
    return nc


_CACHE = {}
LAST_RESULTS = None


def _get_nc():
    if 'nc' not in _CACHE:
        _CACHE['nc'] = _build()
    return _CACHE['nc']


def _pack_x(xc):
    # xc [512, 512, 32] f32 -> two [128, XROWS, UP] f16 class-layout slabs:
    # xp (plain, with halos) and xe (per-row-group u-shift companion:
    # even classes (gw=0) shifted u+1, odd classes (gw=1) shifted u-1).
    xr = xc.reshape(256, 2, 256, 2, 32)            # v gh u gw c
    xr = np.ascontiguousarray(xr.transpose(1, 3, 4, 0, 2))  # gh gw c v u
    xr = xr.reshape(128, 256, 256).astype(np.float16)
    xp = np.zeros((128, XROWS, UP), np.float16)
    xp[:, 2:258, 2:258] = xr
    xe = np.zeros_like(xp)
    even = np.zeros(128, bool)
    for a in range(4):
        if a % 2 == 0:
            even[32 * a:32 * a + 32] = True
    xe[even, :, 0:UP - 1] = xp[even, :, 1:UP]
    xe[~even, :, 1:UP] = xp[~even, :, 0:UP - 1]
    return xp, xe


def kernel(x, w0, b0, w1, b1, w_res):
    from concourse.bass_utils import run_bass_kernel_spmd
    x = np.asarray(x, np.float32)
    wts = _pack_weights(np.asarray(w0), np.asarray(w1), np.asarray(w_res))
    biases = np.zeros((128, 2), np.float32)
    biases[:, 0] = np.tile(np.asarray(b0, np.float32), 4)
    biases[:, 1] = np.tile(np.asarray(b1, np.float32), 4)
    nc = _get_nc()
    in_maps = []
    for i in range(N_CORES):
        xp, xe = _pack_x(x[i])
        in_maps.append({"x": xp, "xe": xe, "wts": wts, "biases": biases})
    res = run_bass_kernel_spmd(nc, in_maps, core_ids=list(range(N_CORES)))
    global LAST_RESULTS
    LAST_RESULTS = res
    outs = []
    for i in range(N_CORES):
        o = res.results[i]["out"]                   # [128, 128, 128]
        o = o.reshape(2, 2, 32, 128, 128)           # oh ow f V U
        o = o.transpose(3, 0, 4, 1, 2).reshape(256, 256, 32)
        outs.append(o)
    return np.stack(outs).astype(np.float32)
